# revision 1
# baseline (speedup 1.0000x reference)
"""Trainium2 Bass kernel for nn_CorrTorch: 27-shift 3D correlation + 1x1x1 conv.

Math (B=1, C=32, D=H=W=64, NOFF=27):
  cv[(k,c), s] = x1[c,s] * pad(x2)[c, s + off_k] / sqrt(C)    (864 x 64^3)
  out[o, s]    = sum_{k,c} conv_w[o, k*32+c] * cv[(k,c), s] + conv_b[o]

Sharding: D axis split across 8 cores (8 planes each), 1-voxel halo baked into
the per-core x2 slab on the host. No collectives.

Per-core device strategy:
  - 3 "replica groups" of 32 channels on partitions 0..95; group g holds data
    pre-shifted by dx=g along W (host-baked), so all 27 shifts reduce to
    9 product passes (dz,dy in {0,1,2}^2), each a single bf16 tensor_tensor
    multiply in 2x mode over 96 partitions (9 passes is provably minimal for
    any fixed-preshift replica scheme).  ~2 of the 9 passes per plane run on
    GPSIMD, in parallel with the vector engine (the bottleneck).
  - The 1x1 conv becomes 9 accumulated matmuls (K=96 contraction chunks) with
    M=27 output channels.  M<32, so 4 spatial subtiles are processed
    concurrently via tile_position column-tiling (col group g4 -> psum
    partitions 32*g4..32*g4+26).
  - ScalarE evicts PSUM -> SBUF fp32 with the conv bias applied per-partition,
    then HWDGE DMA writes straight to HBM.
"""

import numpy as np
import ml_dtypes

import concourse.bass as bass
import concourse.mybir as mybir
import concourse.tile as tile
from concourse.bass_utils import run_bass_kernel_spmd

C = 32
D = 64
H = 64
W = 64
NOFF = 27
NCORES = 8
DLOC = D // NCORES          # 8 output planes per core
NSLAB = DLOC + 2            # 10 padded x2 planes per core
HP = H + 2                  # 66
WP = W + 2                  # 66
PLANE_F = HP * WP           # 4356 elements per padded plane per partition
G = 3                       # dx replica groups
P96 = G * C                 # 96 partitions used by products / contraction
NPASS = 9                   # (dz, dy) passes
TN = H * W                  # 4096 columns per cv tile (one full plane)
SUB = 512                   # columns per matmul (one PSUM bank)
NSUBT = TN // SUB           # 8 spatial subtiles per plane
NSUB = 4                    # col-tiled concurrent matmul groups

BF16 = mybir.dt.bfloat16
F32 = mybir.dt.float32

_wsplit_ctr = [0]


def _split_sync_waits(nc, max_waits=1):
    """Walrus in this container accepts at most one sync wait per instruction.
    Hoist excess waits onto NoOp instructions inserted just before, on the
    same engine (same-engine program order preserves the semantics)."""
    for fn in nc.m.functions:
        for bb in fn.blocks:
            new = []
            changed = False
            for ins in bb.instructions:
                si = ins.sync_info
                if si is not None and len(si.on_wait) > max_waits:
                    waits = list(si.on_wait)
                    excess, keep = waits[:-max_waits], waits[-max_waits:]
                    for i in range(0, len(excess), max_waits):
                        _wsplit_ctr[0] += 1
                        new.append(
                            mybir.InstNoOp(
                                name=f"wsplit-{_wsplit_ctr[0]}",
                                engine=ins.engine,
                                sync_info=mybir.SyncInfo(
                                    on_wait=excess[i : i + max_waits], on_update=[]
                                ),
                            )
                        )
                    ins.sync_info = mybir.SyncInfo(
                        on_wait=keep, on_update=list(si.on_update)
                    )
                    changed = True
                new.append(ins)
            if changed:
                bb.instructions = new


def build_program():
    nc = bass.Bass()

    x1r = nc.dram_tensor("x1r", [DLOC, P96, H * W], BF16, kind="ExternalInput")
    x2r = nc.dram_tensor("x2r", [NSLAB, P96, PLANE_F], BF16, kind="ExternalInput")
    wts = nc.dram_tensor("wts", [P96, NPASS * NOFF], BF16, kind="ExternalInput")
    bias = nc.dram_tensor("bias", [128, 1], F32, kind="ExternalInput")
    out = nc.dram_tensor("out", [NOFF, DLOC * H * W], F32, kind="ExternalOutput")

    with tile.TileContext(nc) as tc:
        with (
            tc.tile_pool(name="wt", bufs=1) as wt_pool,
            tc.tile_pool(name="x2", bufs=5) as x2_pool,
            tc.tile_pool(name="x1", bufs=3) as x1_pool,
            tc.tile_pool(name="cv", bufs=11) as cv_pool,
            tc.tile_pool(name="stage", bufs=3) as stage_pool,
            tc.tile_pool(name="psum", bufs=4, space="PSUM") as psum_pool,
        ):
            x2t = {}

            def load_x2_plane(p):
                t = x2_pool.tile([P96, HP, WP], BF16, tag="x2plane")
                nc.sync.dma_start(out=t[:], in_=x2r[p])
                x2t[p] = t

            # first compute pass needs only x1[0] + x2[0]; issue those first
            x1t0 = x1_pool.tile([P96, H * W], BF16, tag="x1plane")
            nc.sync.dma_start(out=x1t0[:], in_=x1r[0])
            load_x2_plane(0)
            wt_tile = wt_pool.tile([P96, NPASS * NOFF], BF16)
            nc.sync.dma_start(out=wt_tile[:], in_=wts[:])
            bias_tile = wt_pool.tile([128, 1], F32)
            nc.sync.dma_start(out=bias_tile[:], in_=bias[:])
            for p in range(1, 3):
                load_x2_plane(p)

            for d in range(DLOC):
                if d + 3 < NSLAB:
                    load_x2_plane(d + 3)
                if d == 0:
                    x1t = x1t0
                else:
                    x1t = x1_pool.tile([P96, H * W], BF16, tag="x1plane")
                    nc.sync.dma_start(out=x1t[:], in_=x1r[d])

                psums = []
                for _ph in range(2):
                    ps = psum_pool.tile([128, SUB], F32, tag="ps")
                    psums.append(ps)
                for dz in range(3):
                    for dy in range(3):
                        j = 3 * dz + dy
                        cv = cv_pool.tile([P96, TN], BF16, tag="cv")
                        # ~2 passes per plane run on GPSIMD in parallel with
                        # the rest on the (bottleneck) vector engine
                        offload = (d < 7 and (dz, dy) in ((0, 2), (2, 2))) or (
                            d == 7 and (dz, dy) == (2, 2)
                        )
                        eng = nc.gpsimd if offload else nc.vector
                        eng.tensor_mul(
                            out=cv[:],
                            in0=x1t[:],
                            in1=x2t[d + dz][:, dy : dy + H, 0:W],
                        )
                        for s in range(NSUBT):
                            nc.tensor.matmul(
                                psums[s // NSUB][32 * (s % NSUB) : 32 * (s % NSUB) + NOFF, :],
                                lhsT=wt_tile[:, j * NOFF : (j + 1) * NOFF],
                                rhs=cv[:, s * SUB : (s + 1) * SUB],
                                start=(j == 0),
                                stop=(j == NPASS - 1),
                                tile_position=(0, 32 * (s % NSUB)),
                            )
                for half in range(2):
                    stage = stage_pool.tile([128, SUB], F32, tag="stage")
                    nc.scalar.activation(
                        stage[:],
                        psums[half][:],
                        mybir.ActivationFunctionType.Identity,
                        bias=bias_tile[:],
                    )
                    base = d * (H * W) + half * (TN // 2)
                    for g4 in range(NSUB):
                        nc.sync.dma_start(
                            out=out[0:NOFF, base + g4 * SUB : base + (g4 + 1) * SUB],
                            in_=stage[32 * g4 : 32 * g4 + NOFF, :],
                        )

    _split_sync_waits(nc)
    return nc


_PROGRAM = None


def _get_program():
    global _PROGRAM
    if _PROGRAM is None:
        _PROGRAM = build_program()
    return _PROGRAM


def _prep_inputs(in1, in2, conv_w, conv_b):
    """Build the 8 per-core input maps (bf16 layout prep on host)."""
    x1 = np.ascontiguousarray(np.asarray(in1, np.float32).reshape(C, D, H, W))
    x2 = np.ascontiguousarray(np.asarray(in2, np.float32).reshape(C, D, H, W))
    scale = 1.0 / np.sqrt(np.float32(C))
    Wk = (np.asarray(conv_w, np.float32) * scale).reshape(NOFF, NOFF, C)  # [o,k,c]

    wts = np.zeros((P96, NPASS * NOFF), np.float32)
    for dz in range(3):
        for dy in range(3):
            j = 3 * dz + dy
            for g in range(3):
                k = 9 * dz + 3 * dy + g
                wts[32 * g : 32 * g + C, j * NOFF : (j + 1) * NOFF] = Wk[:, k, :].T
    wts = wts.astype(ml_dtypes.bfloat16)

    bias128 = np.zeros((128, 1), np.float32)
    cb = np.asarray(conv_b, np.float32)
    for g4 in range(4):
        bias128[32 * g4 : 32 * g4 + NOFF, 0] = cb

    # Global zero-padded x2: pad plane/row/col index = global index + 1.
    x2p = np.zeros((C, D + 2, HP, WP), np.float32)
    x2p[:, 1 : D + 1, 1 : H + 1, 1 : W + 1] = x2

    in_maps = []
    for m in range(NCORES):
        slab = x2p[:, DLOC * m : DLOC * m + NSLAB]  # [C,10,66,66]
        flat = slab.reshape(C, -1)
        flat = np.concatenate([flat, np.zeros((C, 4), np.float32)], axis=1)
        # replica g = flat shifted by g (dx preshift), cut back to slab planes
        x2rep = np.stack(
            [flat[:, g : g + NSLAB * PLANE_F] for g in range(G)], axis=0
        )  # [3, C, 10*4356]
        x2rep = (
            x2rep.reshape(G * C, NSLAB, PLANE_F)
            .transpose(1, 0, 2)
            .astype(ml_dtypes.bfloat16)
        )  # [10, 96, 4356]

        x1c = x1[:, DLOC * m : DLOC * (m + 1)].reshape(C, -1)  # [C, 8*4096]
        x1rep = (
            np.tile(x1c, (G, 1))
            .reshape(P96, DLOC, H * W)
            .transpose(1, 0, 2)
            .astype(ml_dtypes.bfloat16)
        )  # [8, 96, 4096]

        in_maps.append(
            {
                "x1r": np.ascontiguousarray(x1rep),
                "x2r": np.ascontiguousarray(x2rep),
                "wts": np.ascontiguousarray(wts),
                "bias": bias128,
            }
        )
    return in_maps


def kernel(in1, in2, conv_w, conv_b):
    nc = _get_program()
    in_maps = _prep_inputs(in1, in2, conv_w, conv_b)
    res = run_bass_kernel_spmd(nc, in_maps, core_ids=list(range(NCORES)))
    outs = [r["out"].reshape(NOFF, DLOC, H, W) for r in res.results]
    full = np.concatenate(outs, axis=1)  # [27, 64, 64, 64]
    return full[None].astype(np.float32)  # [1, 27, 64, 64, 64]



# revision 4
# speedup vs baseline: 1.1032x; 1.1032x over previous
"""Trainium2 Bass kernel for nn_CorrTorch: 27-shift 3D correlation + 1x1x1 conv.

Math (B=1, C=32, D=H=W=64, NOFF=27):
  cv[(k,c), s] = x1[c,s] * pad(x2)[c, s + off_k] / sqrt(C)    (864 x 64^3)
  out[o, s]    = sum_{k,c} conv_w[o, k*32+c] * cv[(k,c), s] + conv_b[o]

Sharding: D axis split across 8 cores (8 planes each), halo baked into the
per-core x2 slab on the host. No collectives.

Per-core device strategy (one output plane d at a time):
  - x2 slab planes carry FOUR replica groups of 32 channels on 128 partitions;
    group g is pre-shifted (host-baked) by o_g in (dy,dx):
        O = [(0,0), (-2,0), (-1,-1), (-1,1)]
    With a per-instruction 2D slice offset s, one tensor_tensor multiply
    computes the 4 products {o_g + s} at once.  Exhaustive search over all
    4-offset bakes shows 9 multiplies/plane is the floor (27 shifts, <=4 new
    per instruction, and no 4-cell pattern has >3 disjoint in-cube translates);
    this bake achieves it as 2 fulls + 1 half-width partial per dz:
        dz in {0,2}:  full@s=(2,0), full@s=(2,1), groups{0,1}@s=(2,2)
        dz == 1:      full@s=(1,1), full@s=(2,1), groups{2,3}@s=(3,1)
    covering each of the 27 cells exactly once.
  - The two complementary partials of dz=0/dz=1 stack into ONE 128-row cv
    tile, so the 9 products form only EIGHT matmul contraction chunks
    (7x K=128 + 1x K=64) instead of 9 -> tensor-engine time drops ~11%.
  - ~20 of the 72 multiplies run on GPSIMD via scalar_tensor_tensor
    ((x*1)*y), which maps to a faster Q7 codepath than tensor_tensor mult;
    the rest run on the (bottleneck) vector engine in bf16 2x mode.
  - The 1x1 conv is 8 accumulated matmuls per 512-col subtile with M=32
    (27 outputs + 5 zero rows so PSUM is fully written), 4 spatial subtiles
    packed per PSUM bank via tile_position column tiling.
  - ScalarE evicts PSUM -> SBUF bf16 with the conv bias applied, one DMA per
    PSUM tile writes a packed [128, 512] block to HBM; the host un-packs.
"""

import numpy as np
import ml_dtypes

import concourse.bass as bass
import concourse.mybir as mybir
import concourse.tile as tile
from concourse.alu_op_type import AluOpType
from concourse.bass_utils import run_bass_kernel_spmd

C = 32
D = 64
H = 64
W = 64
NOFF = 27
NCORES = 8
DLOC = D // NCORES          # 8 output planes per core
NSLAB = DLOC + 2            # 10 slab planes per core
SROWS = 67                  # baked slab rows per plane (dyv in [0,67))
SCOLS = 66                  # baked slab cols per plane
PLANE_F = SROWS * SCOLS     # 4422 elements per partition per slab plane
TN = H * W                  # 4096 columns per cv tile (one full plane)
SUB = 512                   # columns per matmul (one PSUM bank)
NSUBT = TN // SUB           # 8 spatial subtiles per plane
NCHUNK = 8                  # matmul contraction chunks per plane
M32 = 32                    # matmul output rows per col group (27 + 5 zero)

# Replica-group pre-shift offsets (dy, dx), searched (see module docstring).
OFFS = [(0, 0), (-2, 0), (-1, -1), (-1, 1)]

# Per-plane chunk specs: (dz, s=(sy,sx), partition ranges with group lists).
# Every chunk is one cv tile; chunk 6 holds two half-width instructions.
# cells(g) = OFFS[g] + s must cover {0,1,2}^2 per dz exactly (checked below).
CHUNKS = [
    (0, [((0, 128), (2, 0))]),
    (0, [((0, 128), (2, 1))]),
    (1, [((0, 128), (1, 1))]),
    (1, [((0, 128), (2, 1))]),
    (2, [((0, 128), (2, 0))]),
    (2, [((0, 128), (2, 1))]),
    (None, [((0, 64), (2, 2), 0), ((64, 128), (3, 1), 1)]),  # dz 0 / dz 1
    (2, [((0, 64), (2, 2))]),
]
CHUNK_K = [128, 128, 128, 128, 128, 128, 128, 64]

BF16 = mybir.dt.bfloat16
F32 = mybir.dt.float32

_wsplit_ctr = [0]


def _split_sync_waits(nc, max_waits=1):
    """Walrus in this container accepts at most one sync wait per instruction.
    Hoist excess waits onto NoOp instructions inserted just before, on the
    same engine (same-engine program order preserves the semantics)."""
    for fn in nc.m.functions:
        for bb in fn.blocks:
            new = []
            changed = False
            for ins in bb.instructions:
                si = ins.sync_info
                if si is not None and len(si.on_wait) > max_waits:
                    waits = list(si.on_wait)
                    excess, keep = waits[:-max_waits], waits[-max_waits:]
                    for i in range(0, len(excess), max_waits):
                        _wsplit_ctr[0] += 1
                        new.append(
                            mybir.InstNoOp(
                                name=f"wsplit-{_wsplit_ctr[0]}",
                                engine=ins.engine,
                                sync_info=mybir.SyncInfo(
                                    on_wait=excess[i : i + max_waits], on_update=[]
                                ),
                            )
                        )
                    ins.sync_info = mybir.SyncInfo(
                        on_wait=keep, on_update=list(si.on_update)
                    )
                    changed = True
                new.append(ins)
            if changed:
                bb.instructions = new


def _chunk_cells():
    """Resolve CHUNKS into per-chunk (dz, cell, group) lists and verify the
    27-cell cover is exact."""
    out = []
    seen = set()
    for ci, (dz0, instrs) in enumerate(CHUNKS):
        cells = []
        for spec in instrs:
            (p0, p1), (sy, sx) = spec[0], spec[1]
            dz = dz0 if len(spec) == 2 else spec[2]
            for g in range(p0 // 32, p1 // 32):
                oy, ox = OFFS[g]
                dy, dx = sy + oy, sx + ox
                if 0 <= dy <= 2 and 0 <= dx <= 2:
                    cell = (dz, dy, dx)
                    assert cell not in seen, (ci, cell)
                    seen.add(cell)
                    cells.append((g, dz, dy, dx))
        out.append(cells)
    assert len(seen) == 27, len(seen)
    return out


_CELLS = _chunk_cells()


def build_program(pool_extra):
    """pool_extra: set of plane indices whose dz=1 partial multiply also runs
    on GPSIMD (the two prefix partials always do)."""
    nc = bass.Bass()

    x1r = nc.dram_tensor("x1r", [DLOC, 128, TN], BF16, kind="ExternalInput")
    x2r = nc.dram_tensor("x2r", [NSLAB, 128, PLANE_F], BF16, kind="ExternalInput")
    wts = nc.dram_tensor("wts", [128, NCHUNK * M32], BF16, kind="ExternalInput")
    bias = nc.dram_tensor("bias", [128, 1], F32, kind="ExternalInput")
    out = nc.dram_tensor("out", [128, DLOC * 2 * SUB], BF16, kind="ExternalOutput")

    with tile.TileContext(nc) as tc:
        with (
            tc.tile_pool(name="wt", bufs=1) as wt_pool,
            tc.tile_pool(name="x2", bufs=5) as x2_pool,
            tc.tile_pool(name="x1", bufs=3) as x1_pool,
            tc.tile_pool(name="cv", bufs=12) as cv_pool,
            tc.tile_pool(name="stage", bufs=3) as stage_pool,
            tc.tile_pool(name="psum", bufs=4, space="PSUM") as psum_pool,
        ):
            x2t = {}

            def load_x2_plane(p):
                t = x2_pool.tile([128, PLANE_F], BF16, tag="x2plane")
                nc.sync.dma_start(out=t[:], in_=x2r[p])
                x2t[p] = t

            x1t0 = x1_pool.tile([128, TN], BF16, tag="x1plane")
            nc.sync.dma_start(out=x1t0[:], in_=x1r[0])
            load_x2_plane(0)
            wt_tile = wt_pool.tile([128, NCHUNK * M32], BF16)
            nc.sync.dma_start(out=wt_tile[:], in_=wts[:])
            bias_tile = wt_pool.tile([128, 1], F32)
            nc.sync.dma_start(out=bias_tile[:], in_=bias[:])
            for p in range(1, 3):
                load_x2_plane(p)

            for d in range(DLOC):
                if d + 3 < NSLAB:
                    load_x2_plane(d + 3)
                if d == 0:
                    x1t = x1t0
                else:
                    x1t = x1_pool.tile([128, TN], BF16, tag="x1plane")
                    nc.sync.dma_start(out=x1t[:], in_=x1r[d])

                # --- 9 multiplies -> 8 cv chunk tiles ---
                cvt = []
                for ci, (dz0, instrs) in enumerate(CHUNKS):
                    cv = cv_pool.tile([128, TN], BF16, tag="cv")
                    for spec in instrs:
                        (p0, p1), (sy, sx) = spec[0], spec[1]
                        dz = dz0 if len(spec) == 2 else spec[2]
                        slab = x2t[d + dz][:].rearrange(
                            "p (y x) -> p y x", y=SROWS, x=SCOLS
                        )
                        in1 = slab[p0:p1, sy : sy + H, sx : sx + W]
                        # partials on GPSIMD (plus dz=1 partial on some planes)
                        on_pool = (ci == 7) or (
                            ci == 6 and (p0 == 0 or d in pool_extra)
                        )
                        if on_pool:
                            nc.gpsimd.scalar_tensor_tensor(
                                out=cv[p0:p1],
                                in0=x1t[p0:p1],
                                scalar=1.0,
                                in1=in1,
                                op0=AluOpType.mult,
                                op1=AluOpType.mult,
                            )
                        else:
                            nc.vector.tensor_mul(
                                out=cv[p0:p1], in0=x1t[p0:p1], in1=in1
                            )
                    cvt.append(cv)

                # --- 8 accumulated matmul chunks, 2 PSUM tiles x 4 col groups ---
                psums = []
                for _ph in range(2):
                    ps = psum_pool.tile([128, SUB], F32, tag="ps")
                    psums.append(ps)
                for ci in range(NCHUNK):
                    kp = CHUNK_K[ci]
                    for s in range(NSUBT):
                        nc.tensor.matmul(
                            psums[s // 4][32 * (s % 4) : 32 * (s % 4) + M32, :],
                            lhsT=wt_tile[0:kp, ci * M32 : (ci + 1) * M32],
                            rhs=cvt[ci][0:kp, s * SUB : (s + 1) * SUB],
                            start=(ci == 0),
                            stop=(ci == NCHUNK - 1),
                            tile_position=(0, 32 * (s % 4)),
                        )

                for half in range(2):
                    stage = stage_pool.tile([128, SUB], BF16, tag="stage")
                    nc.scalar.activation(
                        stage[:],
                        psums[half][:],
                        mybir.ActivationFunctionType.Identity,
                        bias=bias_tile[:],
                    )
                    col = (2 * d + half) * SUB
                    nc.sync.dma_start(
                        out=out[:, col : col + SUB], in_=stage[:]
                    )

    _split_sync_waits(nc)
    return nc


_PROGRAM = None


def _get_program():
    global _PROGRAM
    if _PROGRAM is None:
        _PROGRAM = build_program(pool_extra={2, 5})
    return _PROGRAM


def _prep_inputs(in1, in2, conv_w, conv_b):
    """Build the 8 per-core input maps (bf16 layout prep on host)."""
    x1 = np.ascontiguousarray(np.asarray(in1, np.float32).reshape(C, D, H, W))
    x2 = np.ascontiguousarray(np.asarray(in2, np.float32).reshape(C, D, H, W))
    scale = 1.0 / np.sqrt(np.float32(C))
    Wk = (np.asarray(conv_w, np.float32) * scale).reshape(NOFF, NOFF, C)  # [o,k,c]

    # Weights: [128, 8*32]; row 32g+c, col 32*ci+o = Wk[o, 9dz+3dy+dx, c]
    wts = np.zeros((128, NCHUNK * M32), np.float32)
    for ci, cells in enumerate(_CELLS):
        for (g, dz, dy, dx) in cells:
            k = 9 * dz + 3 * dy + dx
            wts[32 * g : 32 * g + C, ci * M32 : ci * M32 + NOFF] = Wk[:, k, :].T
    wts = wts.astype(ml_dtypes.bfloat16)

    bias128 = np.zeros((128, 1), np.float32)
    cb = np.asarray(conv_b, np.float32)
    for g4 in range(4):
        bias128[32 * g4 : 32 * g4 + NOFF, 0] = cb

    # Globally padded x2 volume: [C, D+2, 66, 66]; plane/row/col = global + 1.
    x2p = np.zeros((C, D + 2, H + 2, W + 2), np.float32)
    x2p[:, 1 : D + 1, 1 : H + 1, 1 : W + 1] = x2

    # Baked slab: buffer_g[dyv, dxv] = x2p_plane[dyv + oy, dxv + ox] (0 outside)
    # so an instruction slice (sy, sx) reads x2p_plane[y + sy + oy, x + sx + ox].
    # Widened scratch plane: rows -2..66 -> 69 (offset +2), cols -1..66 -> 68 (+1).
    in_maps = []
    for m in range(NCORES):
        slab = x2p[:, DLOC * m : DLOC * m + NSLAB]  # [C, 10, 66, 66]
        wide = np.zeros((C, NSLAB, 69, 68), np.float32)
        wide[:, :, 2:68, 1:67] = slab
        x2rep = np.zeros((NSLAB, 128, SROWS, SCOLS), np.float32)
        for g, (oy, ox) in enumerate(OFFS):
            x2rep[:, 32 * g : 32 * g + C] = wide[
                :, :, 2 + oy : 2 + oy + SROWS, 1 + ox : 1 + ox + SCOLS
            ].transpose(1, 0, 2, 3)
        x2rep = x2rep.reshape(NSLAB, 128, PLANE_F).astype(ml_dtypes.bfloat16)

        x1c = x1[:, DLOC * m : DLOC * (m + 1)].reshape(C, DLOC, TN)
        x1rep = (
            np.tile(x1c, (4, 1, 1))
            .reshape(128, DLOC, TN)
            .transpose(1, 0, 2)
            .astype(ml_dtypes.bfloat16)
        )  # [8, 128, 4096]

        in_maps.append(
            {
                "x1r": np.ascontiguousarray(x1rep),
                "x2r": np.ascontiguousarray(x2rep),
                "wts": np.ascontiguousarray(wts),
                "bias": bias128,
            }
        )
    return in_maps


def kernel(in1, in2, conv_w, conv_b):
    nc = _get_program()
    in_maps = _prep_inputs(in1, in2, conv_w, conv_b)
    res = run_bass_kernel_spmd(nc, in_maps, core_ids=list(range(NCORES)))
    outs = []
    for r in res.results:
        # [128, DLOC*2*512] bf16: row 32*g4+o, col (2d+half)*512+c
        a = np.asarray(r["out"], np.float32).reshape(4, 32, DLOC, 2, SUB)
        # -> [o, d, half, g4, c] -> [o, d, 4096]
        core = a.transpose(1, 2, 3, 0, 4).reshape(32, DLOC, TN)[:NOFF]
        outs.append(core.reshape(NOFF, DLOC, H, W))
    full = np.concatenate(outs, axis=1)  # [27, 64, 64, 64]
    return full[None].astype(np.float32)  # [1, 27, 64, 64, 64]
